# revision 12
# baseline (speedup 1.0000x reference)
"""Causal (diagonal=1) attention head for trn2, 8-core SPMD, fp8-hybrid.

Reference computation (fp32):
    k = key @ Wk.T; q = query @ Wq.T; v = value @ Wv.T       # [B,T,H]
    qk = (q @ k.T) / sqrt(E)                                  # [B,T,T]
    qk masked with tril(ones, k=1) and padding_mask           # -inf outside
    attn = softmax(qk, -1) @ v                                # [B,T,H]

Algebraic fold (removes one of three projections):
    qk = query @ M @ key.T   with  M = Wq.T @ Wk / sqrt(E)  (host, fp64)
    attn = softmax-normalized p @ (value @ Wv.T)

Sharding: data-parallel over batch, 2 batches per core, no collectives.

Per-core pipeline (per batch), PSUM always fp32:
    A : qM = query @ (64*M)  fp8 DoubleRow, requant bf16 with scale 1/64.
        Rows t<128 instead use a bf16 matmul (bf16 M) — the softmax of
        early rows has few live keys, so quantization noise there is not
        averaged away; everywhere else fp8 noise cancels across keys.
    B : scores = qM @ keyT   bf16 (exp() amplifies score noise; fp8 here
        fails the 2e-2 gate — measured 4.4e-2 vs 1.1e-2 in simulation)
    p = exp(scores) (ScalarE), causal-zeroed via GPSIMD affine_select
    V : v' = value @ (32*Wv.T) fp8 DoubleRow, requant bf16 scale 1/32;
        rows s<128 via bf16 matmul (same early-row argument).
    C': num = p @ v' ; den = p @ pad01   bf16
    out = num * reciprocal(den)  -> bf16 DMA (cast fp32 on host)

padding_mask is folded in exactly on the host: value rows and the
denominator column are scaled by pad01 = (padding_mask == 0), which
equals softmax with -inf at padded keys.
"""
from contextlib import ExitStack

import numpy as np
import ml_dtypes

import concourse.bass as bass
import concourse.mybir as mybir
import concourse.tile as tile
from concourse.bass_utils import run_bass_kernel_spmd

BF16 = mybir.dt.bfloat16
F8 = mybir.dt.float8e4
F32 = mybir.dt.float32
DR = mybir.MatmulPerfMode.DoubleRow
P = 128
T = 1024           # sequence length
E = 1024           # embed dim
H = 1024           # head dim
NB = 16            # full batch
NCORES = 8
BPC = NB // NCORES  # batches per core
NC = T // P        # 128-chunks per dim (8)

_nc_cache = None


# --- walrus workaround: one sync-wait per instruction ---------------------
def _split_multi_waits(nc):
    """This walrus build rejects instructions with >1 sync wait (2 for
    EventSemaphore).  Move extra waits onto fresh same-engine NOPs placed
    immediately before the instruction; per-engine in-order execution
    preserves the gating, and semaphore updates stay on the original."""
    for fn in nc.m.functions:
        for bb in fn.blocks:
            il = bb.instructions
            idx = 0
            while idx < len(il):
                inst = il[idx]
                si = inst.sync_info
                waits = list(si.on_wait) if si and si.on_wait else []
                cap = 2 if isinstance(inst, mybir.InstEventSemaphore) else 1
                if len(waits) > cap:
                    extra, keep = waits[:-cap], waits[-cap:]
                    for j, w in enumerate(extra):
                        nop = mybir.InstNoOp(
                            name=f"I-wsplit-{inst.name}-{j}",
                            engine=inst.engine,
                            ins=[],
                            outs=[],
                            sync_info=mybir.SyncInfo(on_wait=[w], on_update=[]),
                        )
                        il.insert(idx, nop)
                        idx += 1
                    inst.sync_info = mybir.SyncInfo(
                        on_wait=keep, on_update=list(si.on_update or [])
                    )
                idx += 1


def _emit_batch(nc, pools, b, dram):
    Exp = mybir.ActivationFunctionType.Exp
    Copy = mybir.ActivationFunctionType.Copy
    sb, ps, psd = pools["sb"], pools["ps"], pools["psd"]
    m64, mb, wv32, wvb, q0b = (pools[k] for k in ("m64", "mb", "wv32", "wvb", "q0b"))
    qM = pools["qM"][b]
    pTb, pT8 = pools["pTb"][b], pools["pT8"][b]
    vpb, vp8 = pools["vpb"][b], pools["vp8"][b]

    # -- A: qM = query @ M.  fp8 DoubleRow for cols t>=128; weight DMAs for
    #    batch 0 interleave chunk-by-chunk with the input loads so each
    #    matmul's operands arrive together. --
    qT = sb.tile([P, NC, T], F8, name="qT")
    for ec in range(NC):
        if b == 0:
            nc.sync.dma_start(m64[:, ec, :], dram["m64"][bass.ts(ec, P), :])
        nc.sync.dma_start(qT[:, ec, :], dram["qT"][b, bass.ts(ec, P), :])
    if b == 0:
        for ec in range(NC):
            nc.sync.dma_start(mb[:, ec, :], dram["mb"][bass.ts(ec, P), :])
            nc.sync.dma_start(q0b[:, ec, :], dram["q0b"][bass.ts(ec, P), :])
    for ec2 in range(NC):
        for lo, w in ((P, 384), (512, 512)):
            acc = ps.tile([P, 512], F32, name="ps")
            for j in range(4):
                nc.tensor.matmul(
                    acc[:, :w],
                    lhsT=m64[:, 2 * j:2 * j + 2, bass.ts(ec2, P)],
                    rhs=qT[:, 2 * j:2 * j + 2, lo:lo + w],
                    start=(j == 0),
                    stop=(j == 3),
                    perf_mode=DR,
                )
            nc.scalar.activation(qM[:, ec2, lo:lo + w], acc[:, :w], Copy,
                                 scale=1.0 / 64.0)
    # A-fixup: bf16 qM cols t<128, both batches at once (weights shared)
    if b == 0:
        for ec2 in range(NC):
            acc = ps.tile([P, 512], F32, name="ps")
            for ec1 in range(NC):
                nc.tensor.matmul(
                    acc[:, :BPC * P],
                    lhsT=mb[:, ec1, bass.ts(ec2, P)],
                    rhs=q0b[:, ec1, :],
                    start=(ec1 == 0),
                    stop=(ec1 == NC - 1),
                )
            for bb in range(BPC):
                nc.scalar.copy(pools["qM"][bb][:, ec2, 0:P],
                               acc[:, bb * P:(bb + 1) * P])

    # -- V: v'[s,h] = value @ Wv.T.  fp8 DoubleRow for s>=128.  Runs before
    #    B so the A-fixup's mb-DMA wait and B's kT DMA hide behind it. --
    vT = sb.tile([P, NC, T], F8, name="vT")
    for ec in range(NC):
        if b == 0:
            nc.sync.dma_start(wv32[:, ec, :], dram["wv32"][bass.ts(ec, P), :])
        nc.sync.dma_start(vT[:, ec, :], dram["vT"][b, bass.ts(ec, P), :])
    v0b = sb.tile([P, NC, P], BF16, name="v0b")
    for ec in range(NC):
        if b == 0:
            nc.sync.dma_start(wvb[:, ec, :], dram["wvb"][bass.ts(ec, P), :])
        nc.sync.dma_start(v0b[:, ec, :], dram["v0b"][b, bass.ts(ec, P), :])
    kT = sb.tile([P, NC, T], BF16, name="kT")
    for ec in range(NC):
        nc.sync.dma_start(kT[:, ec, :], dram["kT"][b, bass.ts(ec, P), :])
    padt = sb.tile([P, NC], BF16, name="padt", bufs=2)
    nc.sync.dma_start(
        padt[:], dram["pad"][b].rearrange("(c p) x -> p (c x)", p=P)
    )
    for sc in range(1, NC):
        for hg in range(2):
            acc = ps.tile([P, 512], F32, name="ps")
            for j in range(4):
                nc.tensor.matmul(
                    acc[:],
                    lhsT=vT[:, 2 * j:2 * j + 2, bass.ts(sc, P)],
                    rhs=wv32[:, 2 * j:2 * j + 2, bass.ts(hg, 512)],
                    start=(j == 0),
                    stop=(j == 3),
                    perf_mode=DR,
                )
            if sc < 2:
                dst = vpb[:, sc, bass.ts(hg, 512)]
            else:
                dst = vp8[:, sc - 2, bass.ts(hg, 512)]
            nc.vector.tensor_scalar_mul(dst, acc[:], 1.0 / 32.0)
    # V-fixup: bf16 v' rows s<128
    for hg in range(2):
        acc = ps.tile([P, 512], F32, name="ps")
        for ec in range(NC):
            nc.tensor.matmul(
                acc[:],
                lhsT=v0b[:, ec, :],
                rhs=wvb[:, ec, bass.ts(hg, 512)],
                start=(ec == 0),
                stop=(ec == NC - 1),
            )
        nc.vector.tensor_copy(vpb[:, 0, bass.ts(hg, 512)], acc[:])

    # -- B: scoresT[s,t] = kT-chunk.T @ qM-chunks (bf16), exp, causal zero.
    #    p stored bf16 for s<256 (protects the early-row fixup), fp8 above
    #    (weight noise is self-normalized by den built from the same values).
    #    exp is shifted by -2 so p_max ~ e^4 stays under fp8's 240 max; the
    #    shift cancels exactly in the num/den normalization.  g=1 runs first
    #    so C' tiles 4-7 (whose t-columns live entirely in g=1) can follow
    #    immediately; C' tiles 0-3 run after g=0. --
    denrow = sb.tile([1, T], F32, name="denrow", bufs=2)
    dencol = sb.tile([P, NC], F32, name="dencol", bufs=2)
    rcol = sb.tile([P, NC], F32, name="rcol", bufs=2)
    pools["rcol"] = rcol
    for g in (1, 0):
        nlive = min(4 * (g + 1) + 1, NC)
        dsts = []
        for sc in range(nlive):
            acc = ps.tile([P, 512], F32, name="ps")
            for ec2 in range(NC):
                nc.tensor.matmul(
                    acc[:],
                    lhsT=kT[:, ec2, bass.ts(sc, P)],
                    rhs=qM[:, ec2, bass.ts(g, 512)],
                    start=(ec2 == 0),
                    stop=(ec2 == NC - 1),
                )
            if sc < 2:
                dst = pTb[:, sc, bass.ts(g, 512)]
            else:
                dst = pT8[:, sc - 2, bass.ts(g, 512)]
            nc.scalar.activation(dst, acc[:], Exp, bias=pools["nbias"][:])
            off = P * sc - 512 * g
            if off >= -126:
                # keep where t_local - s_local - off + 1 >= 0 (j <= i+1)
                nc.gpsimd.affine_select(
                    out=dst,
                    in_=dst,
                    compare_op=mybir.AluOpType.is_ge,
                    fill=0.0,
                    base=1 - off,
                    pattern=[[1, 512]],
                    channel_multiplier=-1,
                )
            dsts.append(dst)
        # den row = sum of stored p (post-select) on the PE: 1-partition
        # outputs riding 512-wide streams, replacing 86 one-col matmuls
        # (~13us exposed).  Emitted after the whole block loop so the PE
        # queue never parks waiting for a block's exp+select round-trip.
        denr = psd.tile([1, 512], F32, name="psd")
        for sc, dst in enumerate(dsts):
            nc.tensor.matmul(denr[:], lhsT=padt[:, sc:sc + 1], rhs=dst,
                             start=(sc == 0), stop=(sc == nlive - 1))
        nc.scalar.copy(denrow[:, bass.ts(g, 512)], denr[:])
        # transpose this half of den [1,T] -> [t-partition, chunk] via tiny
        # DMAs, reciprocal, then the C' tiles whose t-columns live in g
        for c in range(4 * g, 4 * g + 4):
            nc.sync.dma_start(dencol[:, c:c + 1], denrow[:, bass.ts(c, P)])
        nc.vector.reciprocal(rcol[:, 4 * g:4 * g + 4],
                             dencol[:, 4 * g:4 * g + 4])
        _emit_cprime(nc, pools, b, dram, range(4 * g, 4 * g + 4))


def _emit_cprime(nc, pools, b, dram, tis):
    Copy = mybir.ActivationFunctionType.Copy
    sb, ps = pools["sb"], pools["ps"]
    pTb, pT8 = pools["pTb"][b], pools["pT8"][b]
    vpb, vp8 = pools["vpb"][b], pools["vp8"][b]
    rcol = pools["rcol"]

    # -- C': attn[t,h] = sum_s p[t,s] v'[s,h]; s<256 bf16, above fp8 pairs --
    for ti in tis:
        nsc = min(ti + 2, NC)
        po0 = ps.tile([P, 512], F32, name="ps")
        po1 = ps.tile([P, 512], F32, name="ps")
        c8 = nsc - 2  # fp8 s-chunks
        for sc in range(min(nsc, 2)):
            lhsT = pTb[:, sc, bass.ts(ti, P)]
            st, sp = (sc == 0), (sc == nsc - 1)
            nc.tensor.matmul(po0[:], lhsT=lhsT, rhs=vpb[:, sc, 0:512],
                             start=st, stop=sp)
            nc.tensor.matmul(po1[:], lhsT=lhsT, rhs=vpb[:, sc, 512:1024],
                             start=st, stop=sp)
        for j in range(c8 // 2):
            lhsT = pT8[:, 2 * j:2 * j + 2, bass.ts(ti, P)]
            sp = (2 * j + 2 == c8)
            nc.tensor.matmul(po0[:], lhsT=lhsT, rhs=vp8[:, 2 * j:2 * j + 2, 0:512],
                             start=False, stop=sp, perf_mode=DR)
            nc.tensor.matmul(po1[:], lhsT=lhsT,
                             rhs=vp8[:, 2 * j:2 * j + 2, 512:1024],
                             start=False, stop=sp, perf_mode=DR)
        if c8 > 0 and c8 % 2:
            lhsT = pT8[:, c8 - 1, bass.ts(ti, P)]
            nc.tensor.matmul(po0[:], lhsT=lhsT, rhs=vp8[:, c8 - 1, 0:512],
                             start=False, stop=True)
            nc.tensor.matmul(po1[:], lhsT=lhsT, rhs=vp8[:, c8 - 1, 512:1024],
                             start=False, stop=True)
        osb = sb.tile([P, T], BF16, name="osb", bufs=3)
        # the two halves scale concurrently on VectorE and ScalarE
        nc.vector.tensor_scalar_mul(osb[:, 0:512], po0[:], rcol[:, ti:ti + 1])
        nc.sync.dma_start(dram["out"][b, bass.ts(ti, P), 0:512], osb[:, 0:512])
        nc.scalar.activation(osb[:, 512:1024], po1[:], Copy,
                             scale=rcol[:, ti:ti + 1])
        nc.sync.dma_start(dram["out"][b, bass.ts(ti, P), 512:1024],
                          osb[:, 512:1024])


def _build_nc():
    nc = bass.Bass()
    dram = {
        "qT": nc.declare_dram_parameter("qT", [BPC, E, T], F8, isOutput=False),
        "kT": nc.declare_dram_parameter("kT", [BPC, E, T], BF16, isOutput=False),
        "vT": nc.declare_dram_parameter("vT", [BPC, E, T], F8, isOutput=False),
        "q0b": nc.declare_dram_parameter("q0b", [E, BPC * P], BF16, isOutput=False),
        "v0b": nc.declare_dram_parameter("v0b", [BPC, E, P], BF16, isOutput=False),
        "m64": nc.declare_dram_parameter("m64", [E, E], F8, isOutput=False),
        "mb": nc.declare_dram_parameter("mb", [E, E], BF16, isOutput=False),
        "wv32": nc.declare_dram_parameter("wv32", [E, H], F8, isOutput=False),
        "wvb": nc.declare_dram_parameter("wvb", [E, H], BF16, isOutput=False),
        "pad": nc.declare_dram_parameter("pad", [BPC, T, 1], BF16, isOutput=False),
        "out": nc.declare_dram_parameter("out", [BPC, T, H], BF16, isOutput=True),
    }
    with tile.TileContext(nc) as tc, ExitStack() as ctx:
        sb = ctx.enter_context(tc.tile_pool(name="sb", bufs=1))
        ps = ctx.enter_context(tc.tile_pool(name="ps", bufs=6, space="PSUM"))
        psd = ctx.enter_context(tc.tile_pool(name="psd", bufs=2, space="PSUM"))

        pools = {"sb": sb, "ps": ps, "psd": psd}
        pools["m64"] = sb.tile([P, NC, E], F8, name="m64")
        pools["mb"] = sb.tile([P, NC, E], BF16, name="mb")
        pools["wv32"] = sb.tile([P, NC, H], F8, name="wv32")
        pools["wvb"] = sb.tile([P, NC, H], BF16, name="wvb")
        pools["q0b"] = sb.tile([P, NC, BPC * P], BF16, name="q0b")
        pools["qM"] = [sb.tile([P, NC, T], BF16, name=f"qM{b}") for b in range(BPC)]
        pools["pTb"] = [sb.tile([P, 2, T], BF16, name=f"pTb{b}") for b in range(BPC)]
        pools["pT8"] = [sb.tile([P, NC - 2, T], F8, name=f"pT8{b}") for b in range(BPC)]
        pools["vpb"] = [sb.tile([P, 2, H], BF16, name=f"vpb{b}") for b in range(BPC)]
        pools["vp8"] = [sb.tile([P, NC - 2, H], F8, name=f"vp8{b}") for b in range(BPC)]

        # PE warm-up: ~3.4us of junk matmuls with no data dependencies fill
        # the startup DMA window and trip the HAM clock gate to 2.4 GHz
        # before the first real matmul arrives.  Output is never read.
        pools["nbias"] = sb.tile([P, 1], F32, name="nbias")
        nc.gpsimd.memset(pools["nbias"][:], -2.0)
        warm = sb.tile([P, 512], BF16, name="warm")
        nc.vector.memset(warm[:], 0.0)
        wps = ps.tile([P, 512], F32, name="ps")
        for _ in range(16):
            nc.tensor.matmul(wps[:], lhsT=warm[:, 0:P], rhs=warm[:],
                             start=True, stop=True)

        for b in range(BPC):
            _emit_batch(nc, pools, b, dram)

    _split_multi_waits(nc)
    return nc


def _get_nc():
    global _nc_cache
    if _nc_cache is None:
        _nc_cache = _build_nc()
    return _nc_cache


def _make_in_maps(key, query, value, padding_mask, Wk, Wq, Wv):
    bf = ml_dtypes.bfloat16
    f8 = ml_dtypes.float8_e4m3
    M = (Wq.T.astype(np.float64) @ Wk.astype(np.float64)
         / np.sqrt(np.float64(E))).astype(np.float32)
    m64 = (M * 64.0).astype(f8)
    mb = M.astype(bf)
    wv32 = (Wv.T * 32.0).astype(f8)
    wvb = np.ascontiguousarray(Wv.T).astype(bf)
    pad01 = (padding_mask.reshape(NB, T) == 0).astype(np.float32)  # [B,T]
    in_maps = []
    for c in range(NCORES):
        s = slice(BPC * c, BPC * (c + 1))
        qTf = query[s].transpose(0, 2, 1)
        kTf = key[s].transpose(0, 2, 1)
        vTf = value[s].transpose(0, 2, 1) * pad01[s][:, None, :]
        q0b = np.ascontiguousarray(
            qTf[:, :, :P].transpose(1, 0, 2).reshape(E, BPC * P)).astype(bf)
        in_maps.append({
            "qT": np.ascontiguousarray(qTf).astype(f8),
            "kT": np.ascontiguousarray(kTf).astype(bf),
            "vT": np.ascontiguousarray(vTf).astype(f8),
            "q0b": q0b,
            "v0b": np.ascontiguousarray(vTf[:, :, :P]).astype(bf),
            "m64": m64, "mb": mb, "wv32": wv32, "wvb": wvb,
            "pad": pad01[s].astype(bf).reshape(BPC, T, 1),
        })
    return in_maps


def run_on_cores(in_maps, trace=False, **kw):
    nc = _get_nc()
    return run_bass_kernel_spmd(nc, in_maps, list(range(NCORES)), trace=trace, **kw)


def kernel(key, query, value, padding_mask, Wk, Wq, Wv):
    key = np.asarray(key)
    query = np.asarray(query)
    value = np.asarray(value)
    padding_mask = np.asarray(padding_mask)
    in_maps = _make_in_maps(key, query, value, padding_mask,
                            np.asarray(Wk), np.asarray(Wq), np.asarray(Wv))
    res = run_on_cores(in_maps)
    out = np.empty((NB, T, H), np.float32)
    for c in range(NCORES):
        out[BPC * c: BPC * (c + 1)] = res.results[c]["out"].astype(np.float32)
    return out


# revision 15
# speedup vs baseline: 1.0447x; 1.0447x over previous
"""Causal (diagonal=1) attention head for trn2, 8-core SPMD, fp8-hybrid.

Reference computation (fp32):
    k = key @ Wk.T; q = query @ Wq.T; v = value @ Wv.T       # [B,T,H]
    qk = (q @ k.T) / sqrt(E)                                  # [B,T,T]
    qk masked with tril(ones, k=1) and padding_mask           # -inf outside
    attn = softmax(qk, -1) @ v                                # [B,T,H]

Algebraic fold (removes one of three projections):
    qk = query @ M @ key.T   with  M = Wq.T @ Wk / sqrt(E)  (host, fp64)
    attn = softmax-normalized p @ (value @ Wv.T)

Sharding: data-parallel over batch, 2 batches per core, no collectives.

Per-core pipeline (per batch), PSUM always fp32:
    A : qM = query @ (64*M)  fp8 DoubleRow, requant bf16 with scale 1/64.
        Rows t<128 instead use a bf16 matmul (bf16 M) — the softmax of
        early rows has few live keys, so quantization noise there is not
        averaged away; everywhere else fp8 noise cancels across keys.
    B : scores = qM @ keyT   bf16 (exp() amplifies score noise; fp8 here
        fails the 2e-2 gate — measured 4.4e-2 vs 1.1e-2 in simulation)
    p = exp(scores) (ScalarE), causal-zeroed via GPSIMD affine_select
    V : v' = value @ (32*Wv.T) fp8 DoubleRow, requant bf16 scale 1/32;
        rows s<128 via bf16 matmul (same early-row argument).
    C': num = p @ v' ; den = p @ pad01   bf16
    out = num * reciprocal(den)  -> bf16 DMA (cast fp32 on host)

padding_mask is folded in exactly on the host: value rows and the
denominator column are scaled by pad01 = (padding_mask == 0), which
equals softmax with -inf at padded keys.
"""
from contextlib import ExitStack

import numpy as np
import ml_dtypes

import concourse.bass as bass
import concourse.mybir as mybir
import concourse.tile as tile
from concourse.bass_utils import run_bass_kernel_spmd

BF16 = mybir.dt.bfloat16
F8 = mybir.dt.float8e4
F32 = mybir.dt.float32
DR = mybir.MatmulPerfMode.DoubleRow
P = 128
T = 1024           # sequence length
E = 1024           # embed dim
H = 1024           # head dim
NB = 16            # full batch
NCORES = 8
BPC = NB // NCORES  # batches per core
NC = T // P        # 128-chunks per dim (8)

_nc_cache = None


# --- walrus workaround: one sync-wait per instruction ---------------------
def _split_multi_waits(nc):
    """This walrus build rejects instructions with >1 sync wait (2 for
    EventSemaphore).  Move extra waits onto fresh same-engine NOPs placed
    immediately before the instruction; per-engine in-order execution
    preserves the gating, and semaphore updates stay on the original."""
    for fn in nc.m.functions:
        for bb in fn.blocks:
            il = bb.instructions
            idx = 0
            while idx < len(il):
                inst = il[idx]
                si = inst.sync_info
                waits = list(si.on_wait) if si and si.on_wait else []
                cap = 2 if isinstance(inst, mybir.InstEventSemaphore) else 1
                if len(waits) > cap:
                    extra, keep = waits[:-cap], waits[-cap:]
                    for j, w in enumerate(extra):
                        nop = mybir.InstNoOp(
                            name=f"I-wsplit-{inst.name}-{j}",
                            engine=inst.engine,
                            ins=[],
                            outs=[],
                            sync_info=mybir.SyncInfo(on_wait=[w], on_update=[]),
                        )
                        il.insert(idx, nop)
                        idx += 1
                    inst.sync_info = mybir.SyncInfo(
                        on_wait=keep, on_update=list(si.on_update or [])
                    )
                idx += 1


def _emit_batch(nc, pools, b, dram):
    Exp = mybir.ActivationFunctionType.Exp
    Copy = mybir.ActivationFunctionType.Copy
    sb, ps, psd = pools["sb"], pools["ps"], pools["psd"]
    m64, mb, wv32, wvb, q0b = (pools[k] for k in ("m64", "mb", "wv32", "wvb", "q0b"))
    qM = pools["qM"][b]
    pTb, pT8 = pools["pTb"][b], pools["pT8"][b]
    vpb, vp8 = pools["vpb"][b], pools["vp8"][b]

    # -- A: qM = query @ M.  fp8 DoubleRow for cols t>=128; weight DMAs for
    #    batch 0 interleave chunk-by-chunk with the input loads so each
    #    matmul's operands arrive together. --
    qT = sb.tile([P, NC, T], F8, name="qT")
    for ec in range(NC):
        if b == 0:
            nc.sync.dma_start(m64[:, ec, :], dram["m64"][bass.ts(ec, P), :])
        nc.sync.dma_start(qT[:, ec, :], dram["qT"][b, bass.ts(ec, P), :])
    if b == 0:
        for ec in range(NC):
            nc.sync.dma_start(mb[:, ec, :], dram["mb"][bass.ts(ec, P), :])
            nc.sync.dma_start(q0b[:, ec, :], dram["q0b"][bass.ts(ec, P), :])
    for ec2 in range(NC):
        for lo, w in ((P, 384), (512, 512)):
            acc = ps.tile([P, 512], F32, name="ps")
            for j in range(4):
                nc.tensor.matmul(
                    acc[:, :w],
                    lhsT=m64[:, 2 * j:2 * j + 2, bass.ts(ec2, P)],
                    rhs=qT[:, 2 * j:2 * j + 2, lo:lo + w],
                    start=(j == 0),
                    stop=(j == 3),
                    perf_mode=DR,
                )
            nc.scalar.activation(qM[:, ec2, lo:lo + w], acc[:, :w], Copy,
                                 scale=1.0 / 64.0)
    def emit_afix():
        # A-fixup: bf16 qM cols t<128, both batches at once (weights shared).
        # Emitted between B's two halves: B(g=1) only reads qM cols 512:1024,
        # so its ~14us of PE work hides the mb/q0b DMA this depends on.
        for ec2 in range(NC):
            acc = ps.tile([P, 512], F32, name="ps")
            for ec1 in range(NC):
                nc.tensor.matmul(
                    acc[:, :BPC * P],
                    lhsT=mb[:, ec1, bass.ts(ec2, P)],
                    rhs=q0b[:, ec1, :],
                    start=(ec1 == 0),
                    stop=(ec1 == NC - 1),
                )
            for bb in range(BPC):
                nc.scalar.copy(pools["qM"][bb][:, ec2, 0:P],
                               acc[:, bb * P:(bb + 1) * P])

    # -- B: scoresT[s,t] = kT-chunk.T @ qM-chunks (bf16), exp, causal zero.
    #    p stored bf16 for s<256 (protects the early-row fixup), fp8 above
    #    (weight noise is self-normalized by den built from the same values).
    #    exp is shifted by -2 so p_max ~ e^4 stays under fp8's 240 max; the
    #    shift cancels exactly in the num/den normalization. --
    kT = sb.tile([P, NC, T], BF16, name="kT")
    for ec in range(NC):
        nc.sync.dma_start(kT[:, ec, :], dram["kT"][b, bass.ts(ec, P), :])
    padt = sb.tile([P, NC], BF16, name="padt", bufs=2)
    nc.sync.dma_start(
        padt[:], dram["pad"][b].rearrange("(c p) x -> p (c x)", p=P)
    )
    denrow = sb.tile([1, T], F32, name="denrow", bufs=2)
    for g in (1, 0):
        nlive = min(4 * (g + 1) + 1, NC)
        dsts = []
        for sc in range(nlive):
            acc = ps.tile([P, 512], F32, name="ps")
            for ec2 in range(NC):
                nc.tensor.matmul(
                    acc[:],
                    lhsT=kT[:, ec2, bass.ts(sc, P)],
                    rhs=qM[:, ec2, bass.ts(g, 512)],
                    start=(ec2 == 0),
                    stop=(ec2 == NC - 1),
                )
            if sc < 2:
                dst = pTb[:, sc, bass.ts(g, 512)]
            else:
                dst = pT8[:, sc - 2, bass.ts(g, 512)]
            nc.scalar.activation(dst, acc[:], Exp, bias=pools["nbias"][:])
            off = P * sc - 512 * g
            if off >= -126:
                # keep where t_local - s_local - off + 1 >= 0 (j <= i+1)
                nc.gpsimd.affine_select(
                    out=dst,
                    in_=dst,
                    compare_op=mybir.AluOpType.is_ge,
                    fill=0.0,
                    base=1 - off,
                    pattern=[[1, 512]],
                    channel_multiplier=-1,
                )
            dsts.append(dst)
        # den row = sum of stored p (post-select) on the PE: 1-partition
        # outputs riding 512-wide streams, replacing 86 one-col matmuls
        # (~13us exposed).  Emitted after the whole block loop so the PE
        # queue never parks waiting for a block's exp+select round-trip.
        denr = psd.tile([1, 512], F32, name="psd")
        for sc, dst in enumerate(dsts):
            nc.tensor.matmul(denr[:], lhsT=padt[:, sc:sc + 1], rhs=dst,
                             start=(sc == 0), stop=(sc == nlive - 1))
        nc.scalar.copy(denrow[:, bass.ts(g, 512)], denr[:])
        if g == 1 and b == 0:
            emit_afix()
    # transpose den [1,T] -> [t-partition, chunk] via 8 tiny DMAs, then recip
    dencol = sb.tile([P, NC], F32, name="dencol", bufs=2)
    for c in range(NC):
        nc.sync.dma_start(dencol[:, c:c + 1], denrow[:, bass.ts(c, P)])
    rcol = sb.tile([P, NC], F32, name="rcol", bufs=2)
    nc.vector.reciprocal(rcol[:], dencol[:])

    # -- V: v'[s,h] = value @ Wv.T.  fp8 DoubleRow for s>=128 --
    vT = sb.tile([P, NC, T], F8, name="vT")
    for ec in range(NC):
        if b == 0:
            nc.sync.dma_start(wv32[:, ec, :], dram["wv32"][bass.ts(ec, P), :])
        nc.sync.dma_start(vT[:, ec, :], dram["vT"][b, bass.ts(ec, P), :])
    v0b = sb.tile([P, NC, P], BF16, name="v0b")
    for ec in range(NC):
        if b == 0:
            nc.sync.dma_start(wvb[:, ec, :], dram["wvb"][bass.ts(ec, P), :])
        nc.sync.dma_start(v0b[:, ec, :], dram["v0b"][b, bass.ts(ec, P), :])
    for sc in range(1, NC):
        for hg in range(2):
            acc = ps.tile([P, 512], F32, name="ps")
            for j in range(4):
                nc.tensor.matmul(
                    acc[:],
                    lhsT=vT[:, 2 * j:2 * j + 2, bass.ts(sc, P)],
                    rhs=wv32[:, 2 * j:2 * j + 2, bass.ts(hg, 512)],
                    start=(j == 0),
                    stop=(j == 3),
                    perf_mode=DR,
                )
            if sc < 2:
                dst = vpb[:, sc, bass.ts(hg, 512)]
            else:
                dst = vp8[:, sc - 2, bass.ts(hg, 512)]
            nc.vector.tensor_scalar_mul(dst, acc[:], 1.0 / 32.0)
    # V-fixup: bf16 v' rows s<128
    for hg in range(2):
        acc = ps.tile([P, 512], F32, name="ps")
        for ec in range(NC):
            nc.tensor.matmul(
                acc[:],
                lhsT=v0b[:, ec, :],
                rhs=wvb[:, ec, bass.ts(hg, 512)],
                start=(ec == 0),
                stop=(ec == NC - 1),
            )
        nc.vector.tensor_copy(vpb[:, 0, bass.ts(hg, 512)], acc[:])

    # -- C': attn[t,h] = sum_s p[t,s] v'[s,h]; s<256 bf16, above fp8 pairs --
    for ti in range(NC):
        nsc = min(ti + 2, NC)
        po0 = ps.tile([P, 512], F32, name="ps")
        po1 = ps.tile([P, 512], F32, name="ps")
        c8 = nsc - 2  # fp8 s-chunks
        for sc in range(min(nsc, 2)):
            lhsT = pTb[:, sc, bass.ts(ti, P)]
            st, sp = (sc == 0), (sc == nsc - 1)
            nc.tensor.matmul(po0[:], lhsT=lhsT, rhs=vpb[:, sc, 0:512],
                             start=st, stop=sp)
            nc.tensor.matmul(po1[:], lhsT=lhsT, rhs=vpb[:, sc, 512:1024],
                             start=st, stop=sp)
        for j in range(c8 // 2):
            lhsT = pT8[:, 2 * j:2 * j + 2, bass.ts(ti, P)]
            sp = (2 * j + 2 == c8)
            nc.tensor.matmul(po0[:], lhsT=lhsT, rhs=vp8[:, 2 * j:2 * j + 2, 0:512],
                             start=False, stop=sp, perf_mode=DR)
            nc.tensor.matmul(po1[:], lhsT=lhsT,
                             rhs=vp8[:, 2 * j:2 * j + 2, 512:1024],
                             start=False, stop=sp, perf_mode=DR)
        if c8 > 0 and c8 % 2:
            lhsT = pT8[:, c8 - 1, bass.ts(ti, P)]
            nc.tensor.matmul(po0[:], lhsT=lhsT, rhs=vp8[:, c8 - 1, 0:512],
                             start=False, stop=True)
            nc.tensor.matmul(po1[:], lhsT=lhsT, rhs=vp8[:, c8 - 1, 512:1024],
                             start=False, stop=True)
        osb = sb.tile([P, T], BF16, name="osb", bufs=3)
        # the two halves scale concurrently on VectorE and ScalarE
        nc.vector.tensor_scalar_mul(osb[:, 0:512], po0[:], rcol[:, ti:ti + 1])
        nc.sync.dma_start(dram["out"][b, bass.ts(ti, P), 0:512], osb[:, 0:512])
        nc.scalar.activation(osb[:, 512:1024], po1[:], Copy,
                             scale=rcol[:, ti:ti + 1])
        nc.sync.dma_start(dram["out"][b, bass.ts(ti, P), 512:1024],
                          osb[:, 512:1024])


def _build_nc():
    nc = bass.Bass()
    dram = {
        "qT": nc.declare_dram_parameter("qT", [BPC, E, T], F8, isOutput=False),
        "kT": nc.declare_dram_parameter("kT", [BPC, E, T], BF16, isOutput=False),
        "vT": nc.declare_dram_parameter("vT", [BPC, E, T], F8, isOutput=False),
        "q0b": nc.declare_dram_parameter("q0b", [E, BPC * P], BF16, isOutput=False),
        "v0b": nc.declare_dram_parameter("v0b", [BPC, E, P], BF16, isOutput=False),
        "m64": nc.declare_dram_parameter("m64", [E, E], F8, isOutput=False),
        "mb": nc.declare_dram_parameter("mb", [E, E], BF16, isOutput=False),
        "wv32": nc.declare_dram_parameter("wv32", [E, H], F8, isOutput=False),
        "wvb": nc.declare_dram_parameter("wvb", [E, H], BF16, isOutput=False),
        "pad": nc.declare_dram_parameter("pad", [BPC, T, 1], BF16, isOutput=False),
        "out": nc.declare_dram_parameter("out", [BPC, T, H], BF16, isOutput=True),
    }
    with tile.TileContext(nc) as tc, ExitStack() as ctx:
        sb = ctx.enter_context(tc.tile_pool(name="sb", bufs=1))
        ps = ctx.enter_context(tc.tile_pool(name="ps", bufs=6, space="PSUM"))
        psd = ctx.enter_context(tc.tile_pool(name="psd", bufs=2, space="PSUM"))

        pools = {"sb": sb, "ps": ps, "psd": psd}
        pools["m64"] = sb.tile([P, NC, E], F8, name="m64")
        pools["mb"] = sb.tile([P, NC, E], BF16, name="mb")
        pools["wv32"] = sb.tile([P, NC, H], F8, name="wv32")
        pools["wvb"] = sb.tile([P, NC, H], BF16, name="wvb")
        pools["q0b"] = sb.tile([P, NC, BPC * P], BF16, name="q0b")
        pools["qM"] = [sb.tile([P, NC, T], BF16, name=f"qM{b}") for b in range(BPC)]
        pools["pTb"] = [sb.tile([P, 2, T], BF16, name=f"pTb{b}") for b in range(BPC)]
        pools["pT8"] = [sb.tile([P, NC - 2, T], F8, name=f"pT8{b}") for b in range(BPC)]
        pools["vpb"] = [sb.tile([P, 2, H], BF16, name=f"vpb{b}") for b in range(BPC)]
        pools["vp8"] = [sb.tile([P, NC - 2, H], F8, name=f"vp8{b}") for b in range(BPC)]

        # PE warm-up: ~3.4us of junk matmuls with no data dependencies fill
        # the startup DMA window and trip the HAM clock gate to 2.4 GHz
        # before the first real matmul arrives.  Output is never read.
        pools["nbias"] = sb.tile([P, 1], F32, name="nbias")
        nc.gpsimd.memset(pools["nbias"][:], -2.0)
        warm = sb.tile([P, 512], BF16, name="warm")
        nc.vector.memset(warm[:], 0.0)
        wps = ps.tile([P, 512], F32, name="ps")
        for _ in range(16):
            nc.tensor.matmul(wps[:], lhsT=warm[:, 0:P], rhs=warm[:],
                             start=True, stop=True)

        for b in range(BPC):
            _emit_batch(nc, pools, b, dram)

    _split_multi_waits(nc)
    return nc


def _get_nc():
    global _nc_cache
    if _nc_cache is None:
        _nc_cache = _build_nc()
    return _nc_cache


def _make_in_maps(key, query, value, padding_mask, Wk, Wq, Wv):
    bf = ml_dtypes.bfloat16
    f8 = ml_dtypes.float8_e4m3
    M = (Wq.T.astype(np.float64) @ Wk.astype(np.float64)
         / np.sqrt(np.float64(E))).astype(np.float32)
    m64 = (M * 64.0).astype(f8)
    mb = M.astype(bf)
    wv32 = (Wv.T * 32.0).astype(f8)
    wvb = np.ascontiguousarray(Wv.T).astype(bf)
    pad01 = (padding_mask.reshape(NB, T) == 0).astype(np.float32)  # [B,T]
    in_maps = []
    for c in range(NCORES):
        s = slice(BPC * c, BPC * (c + 1))
        qTf = query[s].transpose(0, 2, 1)
        kTf = key[s].transpose(0, 2, 1)
        vTf = value[s].transpose(0, 2, 1) * pad01[s][:, None, :]
        q0b = np.ascontiguousarray(
            qTf[:, :, :P].transpose(1, 0, 2).reshape(E, BPC * P)).astype(bf)
        in_maps.append({
            "qT": np.ascontiguousarray(qTf).astype(f8),
            "kT": np.ascontiguousarray(kTf).astype(bf),
            "vT": np.ascontiguousarray(vTf).astype(f8),
            "q0b": q0b,
            "v0b": np.ascontiguousarray(vTf[:, :, :P]).astype(bf),
            "m64": m64, "mb": mb, "wv32": wv32, "wvb": wvb,
            "pad": pad01[s].astype(bf).reshape(BPC, T, 1),
        })
    return in_maps


def run_on_cores(in_maps, trace=False, **kw):
    nc = _get_nc()
    return run_bass_kernel_spmd(nc, in_maps, list(range(NCORES)), trace=trace, **kw)


def kernel(key, query, value, padding_mask, Wk, Wq, Wv):
    key = np.asarray(key)
    query = np.asarray(query)
    value = np.asarray(value)
    padding_mask = np.asarray(padding_mask)
    in_maps = _make_in_maps(key, query, value, padding_mask,
                            np.asarray(Wk), np.asarray(Wq), np.asarray(Wv))
    res = run_on_cores(in_maps)
    out = np.empty((NB, T, H), np.float32)
    for c in range(NCORES):
        out[BPC * c: BPC * (c + 1)] = res.results[c]["out"].astype(np.float32)
    return out


# revision 17
# speedup vs baseline: 1.0895x; 1.0428x over previous
"""Causal (diagonal=1) attention head for trn2, 8-core SPMD, fp8-hybrid.

Reference computation (fp32):
    k = key @ Wk.T; q = query @ Wq.T; v = value @ Wv.T       # [B,T,H]
    qk = (q @ k.T) / sqrt(E)                                  # [B,T,T]
    qk masked with tril(ones, k=1) and padding_mask           # -inf outside
    attn = softmax(qk, -1) @ v                                # [B,T,H]

Algebraic fold (removes one of three projections):
    qk = query @ M @ key.T   with  M = Wq.T @ Wk / sqrt(E)  (host, fp64)
    attn = softmax-normalized p @ (value @ Wv.T)

Sharding: data-parallel over batch, 2 batches per core, no collectives.

Per-core pipeline (per batch), PSUM always fp32:
    A : qM = query @ (64*M)  fp8 DoubleRow, requant bf16 with scale 1/64.
        Rows t<128 instead use a bf16 matmul (bf16 M) — the softmax of
        early rows has few live keys, so quantization noise there is not
        averaged away; everywhere else fp8 noise cancels across keys.
    B : scores = qM @ keyT   bf16 (exp() amplifies score noise; fp8 here
        fails the 2e-2 gate — measured 4.4e-2 vs 1.1e-2 in simulation)
    p = exp(scores) (ScalarE), causal-zeroed via GPSIMD affine_select
    V : v' = value @ (32*Wv.T) fp8 DoubleRow, requant bf16 scale 1/32;
        rows s<128 via bf16 matmul (same early-row argument).
    C': num = p @ v' ; den = p @ pad01   bf16
    out = num * reciprocal(den)  -> bf16 DMA (cast fp32 on host)

padding_mask is folded in exactly on the host: value rows and the
denominator column are scaled by pad01 = (padding_mask == 0), which
equals softmax with -inf at padded keys.
"""
from contextlib import ExitStack

import numpy as np
import ml_dtypes

import concourse.bass as bass
import concourse.mybir as mybir
import concourse.tile as tile
from concourse.bass_utils import run_bass_kernel_spmd

BF16 = mybir.dt.bfloat16
F8 = mybir.dt.float8e4
F32 = mybir.dt.float32
DR = mybir.MatmulPerfMode.DoubleRow
P = 128
T = 1024           # sequence length
E = 1024           # embed dim
H = 1024           # head dim
NB = 16            # full batch
NCORES = 8
BPC = NB // NCORES  # batches per core
NC = T // P        # 128-chunks per dim (8)

_nc_cache = None


# --- walrus workaround: one sync-wait per instruction ---------------------
def _split_multi_waits(nc):
    """This walrus build rejects instructions with >1 sync wait (2 for
    EventSemaphore).  Move extra waits onto fresh same-engine NOPs placed
    immediately before the instruction; per-engine in-order execution
    preserves the gating, and semaphore updates stay on the original."""
    for fn in nc.m.functions:
        for bb in fn.blocks:
            il = bb.instructions
            idx = 0
            while idx < len(il):
                inst = il[idx]
                si = inst.sync_info
                waits = list(si.on_wait) if si and si.on_wait else []
                cap = 2 if isinstance(inst, mybir.InstEventSemaphore) else 1
                if len(waits) > cap:
                    extra, keep = waits[:-cap], waits[-cap:]
                    for j, w in enumerate(extra):
                        nop = mybir.InstNoOp(
                            name=f"I-wsplit-{inst.name}-{j}",
                            engine=inst.engine,
                            ins=[],
                            outs=[],
                            sync_info=mybir.SyncInfo(on_wait=[w], on_update=[]),
                        )
                        il.insert(idx, nop)
                        idx += 1
                    inst.sync_info = mybir.SyncInfo(
                        on_wait=keep, on_update=list(si.on_update or [])
                    )
                idx += 1


def _emit_batch(nc, pools, b, dram):
    Exp = mybir.ActivationFunctionType.Exp
    Copy = mybir.ActivationFunctionType.Copy
    sb, ps, psd = pools["sb"], pools["ps"], pools["psd"]
    m64, mb, wv32, wvb, q0b = (pools[k] for k in ("m64", "mb", "wv32", "wvb", "q0b"))
    qM = pools["qM"][b]
    pTb, pT8 = pools["pTb"][b], pools["pT8"][b]
    vpb, vp8 = pools["vpb"][b], pools["vp8"][b]

    # -- A: qM = query @ M.  fp8 DoubleRow for cols t>=128; weight DMAs for
    #    batch 0 interleave chunk-by-chunk with the input loads so each
    #    matmul's operands arrive together. --
    qT = sb.tile([P, NC, T], F8, name="qT")
    for ec in range(NC):
        if b == 0:
            nc.sync.dma_start(m64[:, ec, :], dram["m64"][bass.ts(ec, P), :])
        nc.sync.dma_start(qT[:, ec, :], dram["qT"][b, bass.ts(ec, P), :])
    if b == 0:
        for ec in range(NC):
            nc.sync.dma_start(mb[:, ec, :], dram["mb"][bass.ts(ec, P), :])
            nc.sync.dma_start(q0b[:, ec, :], dram["q0b"][bass.ts(ec, P), :])
    for ec2 in range(NC):
        for lo, w in ((P, 384), (512, 512)):
            acc = ps.tile([P, 512], F32, name="ps")
            for j in range(4):
                nc.tensor.matmul(
                    acc[:, :w],
                    lhsT=m64[:, 2 * j:2 * j + 2, bass.ts(ec2, P)],
                    rhs=qT[:, 2 * j:2 * j + 2, lo:lo + w],
                    start=(j == 0),
                    stop=(j == 3),
                    perf_mode=DR,
                )
            nc.scalar.activation(qM[:, ec2, lo:lo + w], acc[:, :w], Copy,
                                 scale=1.0 / 64.0)
    # A-fixup: bf16 qM cols t<128, both batches at once (weights shared)
    if b == 0:
        for ec2 in range(NC):
            acc = ps.tile([P, 512], F32, name="ps")
            for ec1 in range(NC):
                nc.tensor.matmul(
                    acc[:, :BPC * P],
                    lhsT=mb[:, ec1, bass.ts(ec2, P)],
                    rhs=q0b[:, ec1, :],
                    start=(ec1 == 0),
                    stop=(ec1 == NC - 1),
                )
            for bb in range(BPC):
                nc.scalar.copy(pools["qM"][bb][:, ec2, 0:P],
                               acc[:, bb * P:(bb + 1) * P])

    # -- B: scoresT[s,t] = kT-chunk.T @ qM-chunks (bf16), exp, causal zero.
    #    p stored bf16 for s<256 (protects the early-row fixup), fp8 above
    #    (weight noise is self-normalized by den built from the same values).
    #    exp is shifted by -2 so p_max ~ e^4 stays under fp8's 240 max; the
    #    shift cancels exactly in the num/den normalization. --
    kT = sb.tile([P, NC, T], BF16, name="kT")
    for ec in range(NC):
        nc.sync.dma_start(kT[:, ec, :], dram["kT"][b, bass.ts(ec, P), :])
    padt = sb.tile([P, NC], BF16, name="padt", bufs=2)
    nc.sync.dma_start(
        padt[:], dram["pad"][b].rearrange("(c p) x -> p (c x)", p=P)
    )
    # 256-wide t-groups: 23 causal-live [128s x 256t] blocks vs 13 512-wide
    # ones (26 512-equivalents) — ~12% less B stream time.
    GW = 256
    denrow = sb.tile([1, T], F32, name="denrow", bufs=2)
    for g in range(T // GW):
        nlive = min(2 * (g + 1) + 1, NC)
        dsts = []
        for sc in range(nlive):
            acc = ps.tile([P, GW], F32, name="ps")
            for ec2 in range(NC):
                nc.tensor.matmul(
                    acc[:],
                    lhsT=kT[:, ec2, bass.ts(sc, P)],
                    rhs=qM[:, ec2, bass.ts(g, GW)],
                    start=(ec2 == 0),
                    stop=(ec2 == NC - 1),
                )
            if sc < 2:
                dst = pTb[:, sc, bass.ts(g, GW)]
            else:
                dst = pT8[:, sc - 2, bass.ts(g, GW)]
            nc.scalar.activation(dst, acc[:], Exp, bias=pools["nbias"][:])
            off = P * sc - GW * g
            if off >= -126:
                # keep where t_local - s_local - off + 1 >= 0 (j <= i+1)
                nc.gpsimd.affine_select(
                    out=dst,
                    in_=dst,
                    compare_op=mybir.AluOpType.is_ge,
                    fill=0.0,
                    base=1 - off,
                    pattern=[[1, GW]],
                    channel_multiplier=-1,
                )
            dsts.append(dst)
        # den row = sum of stored p (post-select) on the PE: 1-partition
        # outputs riding wide streams, replacing 86 one-col matmuls
        # (~13us exposed).  Emitted after the whole block loop so the PE
        # queue never parks waiting for a block's exp+select round-trip.
        denr = psd.tile([1, GW], F32, name="psd")
        for sc, dst in enumerate(dsts):
            nc.tensor.matmul(denr[:], lhsT=padt[:, sc:sc + 1], rhs=dst,
                             start=(sc == 0), stop=(sc == nlive - 1))
        nc.scalar.copy(denrow[:, bass.ts(g, GW)], denr[:])
    # transpose den [1,T] -> [t-partition, chunk] via 8 tiny DMAs, then recip
    dencol = sb.tile([P, NC], F32, name="dencol", bufs=2)
    for c in range(NC):
        nc.sync.dma_start(dencol[:, c:c + 1], denrow[:, bass.ts(c, P)])
    rcol = sb.tile([P, NC], F32, name="rcol", bufs=2)
    nc.vector.reciprocal(rcol[:], dencol[:])

    # -- V: v'[s,h] = value @ Wv.T.  fp8 DoubleRow for s>=128 --
    vT = sb.tile([P, NC, T], F8, name="vT")
    for ec in range(NC):
        if b == 0:
            nc.sync.dma_start(wv32[:, ec, :], dram["wv32"][bass.ts(ec, P), :])
        nc.sync.dma_start(vT[:, ec, :], dram["vT"][b, bass.ts(ec, P), :])
    v0b = sb.tile([P, NC, P], BF16, name="v0b")
    for ec in range(NC):
        if b == 0:
            nc.sync.dma_start(wvb[:, ec, :], dram["wvb"][bass.ts(ec, P), :])
        nc.sync.dma_start(v0b[:, ec, :], dram["v0b"][b, bass.ts(ec, P), :])
    for sc in range(1, NC):
        for hg in range(2):
            acc = ps.tile([P, 512], F32, name="ps")
            for j in range(4):
                nc.tensor.matmul(
                    acc[:],
                    lhsT=vT[:, 2 * j:2 * j + 2, bass.ts(sc, P)],
                    rhs=wv32[:, 2 * j:2 * j + 2, bass.ts(hg, 512)],
                    start=(j == 0),
                    stop=(j == 3),
                    perf_mode=DR,
                )
            if sc < 2:
                dst = vpb[:, sc, bass.ts(hg, 512)]
            else:
                dst = vp8[:, sc - 2, bass.ts(hg, 512)]
            nc.vector.tensor_scalar_mul(dst, acc[:], 1.0 / 32.0)
    # V-fixup: bf16 v' rows s<128
    for hg in range(2):
        acc = ps.tile([P, 512], F32, name="ps")
        for ec in range(NC):
            nc.tensor.matmul(
                acc[:],
                lhsT=v0b[:, ec, :],
                rhs=wvb[:, ec, bass.ts(hg, 512)],
                start=(ec == 0),
                stop=(ec == NC - 1),
            )
        nc.vector.tensor_copy(vpb[:, 0, bass.ts(hg, 512)], acc[:])

    # -- C': attn[t,h] = sum_s p[t,s] v'[s,h]; s<256 bf16, above fp8 pairs --
    for ti in range(NC):
        nsc = min(ti + 2, NC)
        po0 = ps.tile([P, 512], F32, name="ps")
        po1 = ps.tile([P, 512], F32, name="ps")
        c8 = nsc - 2  # fp8 s-chunks
        for sc in range(min(nsc, 2)):
            lhsT = pTb[:, sc, bass.ts(ti, P)]
            st, sp = (sc == 0), (sc == nsc - 1)
            nc.tensor.matmul(po0[:], lhsT=lhsT, rhs=vpb[:, sc, 0:512],
                             start=st, stop=sp)
            nc.tensor.matmul(po1[:], lhsT=lhsT, rhs=vpb[:, sc, 512:1024],
                             start=st, stop=sp)
        for j in range(c8 // 2):
            lhsT = pT8[:, 2 * j:2 * j + 2, bass.ts(ti, P)]
            sp = (2 * j + 2 == c8)
            nc.tensor.matmul(po0[:], lhsT=lhsT, rhs=vp8[:, 2 * j:2 * j + 2, 0:512],
                             start=False, stop=sp, perf_mode=DR)
            nc.tensor.matmul(po1[:], lhsT=lhsT,
                             rhs=vp8[:, 2 * j:2 * j + 2, 512:1024],
                             start=False, stop=sp, perf_mode=DR)
        if c8 > 0 and c8 % 2:
            lhsT = pT8[:, c8 - 1, bass.ts(ti, P)]
            nc.tensor.matmul(po0[:], lhsT=lhsT, rhs=vp8[:, c8 - 1, 0:512],
                             start=False, stop=True)
            nc.tensor.matmul(po1[:], lhsT=lhsT, rhs=vp8[:, c8 - 1, 512:1024],
                             start=False, stop=True)
        osb = sb.tile([P, T], BF16, name="osb", bufs=3)
        # the two halves scale concurrently on VectorE and ScalarE
        nc.vector.tensor_scalar_mul(osb[:, 0:512], po0[:], rcol[:, ti:ti + 1])
        nc.sync.dma_start(dram["out"][b, bass.ts(ti, P), 0:512], osb[:, 0:512])
        nc.scalar.activation(osb[:, 512:1024], po1[:], Copy,
                             scale=rcol[:, ti:ti + 1])
        nc.sync.dma_start(dram["out"][b, bass.ts(ti, P), 512:1024],
                          osb[:, 512:1024])


def _build_nc():
    nc = bass.Bass()
    dram = {
        "qT": nc.declare_dram_parameter("qT", [BPC, E, T], F8, isOutput=False),
        "kT": nc.declare_dram_parameter("kT", [BPC, E, T], BF16, isOutput=False),
        "vT": nc.declare_dram_parameter("vT", [BPC, E, T], F8, isOutput=False),
        "q0b": nc.declare_dram_parameter("q0b", [E, BPC * P], BF16, isOutput=False),
        "v0b": nc.declare_dram_parameter("v0b", [BPC, E, P], BF16, isOutput=False),
        "m64": nc.declare_dram_parameter("m64", [E, E], F8, isOutput=False),
        "mb": nc.declare_dram_parameter("mb", [E, E], BF16, isOutput=False),
        "wv32": nc.declare_dram_parameter("wv32", [E, H], F8, isOutput=False),
        "wvb": nc.declare_dram_parameter("wvb", [E, H], BF16, isOutput=False),
        "pad": nc.declare_dram_parameter("pad", [BPC, T, 1], BF16, isOutput=False),
        "out": nc.declare_dram_parameter("out", [BPC, T, H], BF16, isOutput=True),
    }
    with tile.TileContext(nc) as tc, ExitStack() as ctx:
        sb = ctx.enter_context(tc.tile_pool(name="sb", bufs=1))
        ps = ctx.enter_context(tc.tile_pool(name="ps", bufs=6, space="PSUM"))
        psd = ctx.enter_context(tc.tile_pool(name="psd", bufs=2, space="PSUM"))

        pools = {"sb": sb, "ps": ps, "psd": psd}
        pools["m64"] = sb.tile([P, NC, E], F8, name="m64")
        pools["mb"] = sb.tile([P, NC, E], BF16, name="mb")
        pools["wv32"] = sb.tile([P, NC, H], F8, name="wv32")
        pools["wvb"] = sb.tile([P, NC, H], BF16, name="wvb")
        pools["q0b"] = sb.tile([P, NC, BPC * P], BF16, name="q0b")
        pools["qM"] = [sb.tile([P, NC, T], BF16, name=f"qM{b}") for b in range(BPC)]
        pools["pTb"] = [sb.tile([P, 2, T], BF16, name=f"pTb{b}") for b in range(BPC)]
        pools["pT8"] = [sb.tile([P, NC - 2, T], F8, name=f"pT8{b}") for b in range(BPC)]
        pools["vpb"] = [sb.tile([P, 2, H], BF16, name=f"vpb{b}") for b in range(BPC)]
        pools["vp8"] = [sb.tile([P, NC - 2, H], F8, name=f"vp8{b}") for b in range(BPC)]

        # PE warm-up: ~3.4us of junk matmuls with no data dependencies fill
        # the startup DMA window and trip the HAM clock gate to 2.4 GHz
        # before the first real matmul arrives.  Output is never read.
        pools["nbias"] = sb.tile([P, 1], F32, name="nbias")
        nc.gpsimd.memset(pools["nbias"][:], -2.0)
        warm = sb.tile([P, 512], BF16, name="warm")
        nc.vector.memset(warm[:], 0.0)
        wps = ps.tile([P, 512], F32, name="ps")
        for _ in range(16):
            nc.tensor.matmul(wps[:], lhsT=warm[:, 0:P], rhs=warm[:],
                             start=True, stop=True)

        for b in range(BPC):
            _emit_batch(nc, pools, b, dram)

    _split_multi_waits(nc)
    return nc


def _get_nc():
    global _nc_cache
    if _nc_cache is None:
        _nc_cache = _build_nc()
    return _nc_cache


def _make_in_maps(key, query, value, padding_mask, Wk, Wq, Wv):
    bf = ml_dtypes.bfloat16
    f8 = ml_dtypes.float8_e4m3
    M = (Wq.T.astype(np.float64) @ Wk.astype(np.float64)
         / np.sqrt(np.float64(E))).astype(np.float32)
    m64 = (M * 64.0).astype(f8)
    mb = M.astype(bf)
    wv32 = (Wv.T * 32.0).astype(f8)
    wvb = np.ascontiguousarray(Wv.T).astype(bf)
    pad01 = (padding_mask.reshape(NB, T) == 0).astype(np.float32)  # [B,T]
    in_maps = []
    for c in range(NCORES):
        s = slice(BPC * c, BPC * (c + 1))
        qTf = query[s].transpose(0, 2, 1)
        kTf = key[s].transpose(0, 2, 1)
        vTf = value[s].transpose(0, 2, 1) * pad01[s][:, None, :]
        q0b = np.ascontiguousarray(
            qTf[:, :, :P].transpose(1, 0, 2).reshape(E, BPC * P)).astype(bf)
        in_maps.append({
            "qT": np.ascontiguousarray(qTf).astype(f8),
            "kT": np.ascontiguousarray(kTf).astype(bf),
            "vT": np.ascontiguousarray(vTf).astype(f8),
            "q0b": q0b,
            "v0b": np.ascontiguousarray(vTf[:, :, :P]).astype(bf),
            "m64": m64, "mb": mb, "wv32": wv32, "wvb": wvb,
            "pad": pad01[s].astype(bf).reshape(BPC, T, 1),
        })
    return in_maps


def run_on_cores(in_maps, trace=False, **kw):
    nc = _get_nc()
    return run_bass_kernel_spmd(nc, in_maps, list(range(NCORES)), trace=trace, **kw)


def kernel(key, query, value, padding_mask, Wk, Wq, Wv):
    key = np.asarray(key)
    query = np.asarray(query)
    value = np.asarray(value)
    padding_mask = np.asarray(padding_mask)
    in_maps = _make_in_maps(key, query, value, padding_mask,
                            np.asarray(Wk), np.asarray(Wq), np.asarray(Wv))
    res = run_on_cores(in_maps)
    out = np.empty((NB, T, H), np.float32)
    for c in range(NCORES):
        out[BPC * c: BPC * (c + 1)] = res.results[c]["out"].astype(np.float32)
    return out


# revision 18
# speedup vs baseline: 1.0943x; 1.0044x over previous
"""Causal (diagonal=1) attention head for trn2, 8-core SPMD, fp8-hybrid.

Reference computation (fp32):
    k = key @ Wk.T; q = query @ Wq.T; v = value @ Wv.T       # [B,T,H]
    qk = (q @ k.T) / sqrt(E)                                  # [B,T,T]
    qk masked with tril(ones, k=1) and padding_mask           # -inf outside
    attn = softmax(qk, -1) @ v                                # [B,T,H]

Algebraic fold (removes one of three projections):
    qk = query @ M @ key.T   with  M = Wq.T @ Wk / sqrt(E)  (host, fp64)
    attn = softmax-normalized p @ (value @ Wv.T)

Sharding: data-parallel over batch, 2 batches per core, no collectives.

Per-core pipeline (per batch), PSUM always fp32:
    A : qM = query @ (64*M)  fp8 DoubleRow, requant bf16 with scale 1/64.
        Rows t<128 instead use a bf16 matmul (bf16 M) — the softmax of
        early rows has few live keys, so quantization noise there is not
        averaged away; everywhere else fp8 noise cancels across keys.
    B : scores = qM @ keyT   bf16 (exp() amplifies score noise; fp8 here
        fails the 2e-2 gate — measured 4.4e-2 vs 1.1e-2 in simulation)
    p = exp(scores) (ScalarE), causal-zeroed via GPSIMD affine_select
    V : v' = value @ (32*Wv.T) fp8 DoubleRow, requant bf16 scale 1/32;
        rows s<128 via bf16 matmul (same early-row argument).
    C': num = p @ v' ; den = p @ pad01   bf16
    out = num * reciprocal(den)  -> bf16 DMA (cast fp32 on host)

padding_mask is folded in exactly on the host: value rows and the
denominator column are scaled by pad01 = (padding_mask == 0), which
equals softmax with -inf at padded keys.
"""
from contextlib import ExitStack

import numpy as np
import ml_dtypes

import concourse.bass as bass
import concourse.mybir as mybir
import concourse.tile as tile
from concourse.bass_utils import run_bass_kernel_spmd

BF16 = mybir.dt.bfloat16
F8 = mybir.dt.float8e4
F32 = mybir.dt.float32
DR = mybir.MatmulPerfMode.DoubleRow
P = 128
T = 1024           # sequence length
E = 1024           # embed dim
H = 1024           # head dim
NB = 16            # full batch
NCORES = 8
BPC = NB // NCORES  # batches per core
NC = T // P        # 128-chunks per dim (8)

_nc_cache = None


# --- walrus workaround: one sync-wait per instruction ---------------------
def _split_multi_waits(nc):
    """This walrus build rejects instructions with >1 sync wait (2 for
    EventSemaphore).  Move extra waits onto fresh same-engine NOPs placed
    immediately before the instruction; per-engine in-order execution
    preserves the gating, and semaphore updates stay on the original."""
    for fn in nc.m.functions:
        for bb in fn.blocks:
            il = bb.instructions
            idx = 0
            while idx < len(il):
                inst = il[idx]
                si = inst.sync_info
                waits = list(si.on_wait) if si and si.on_wait else []
                cap = 2 if isinstance(inst, mybir.InstEventSemaphore) else 1
                if len(waits) > cap:
                    extra, keep = waits[:-cap], waits[-cap:]
                    for j, w in enumerate(extra):
                        nop = mybir.InstNoOp(
                            name=f"I-wsplit-{inst.name}-{j}",
                            engine=inst.engine,
                            ins=[],
                            outs=[],
                            sync_info=mybir.SyncInfo(on_wait=[w], on_update=[]),
                        )
                        il.insert(idx, nop)
                        idx += 1
                    inst.sync_info = mybir.SyncInfo(
                        on_wait=keep, on_update=list(si.on_update or [])
                    )
                idx += 1


def _emit_batch(nc, pools, b, dram):
    Exp = mybir.ActivationFunctionType.Exp
    Copy = mybir.ActivationFunctionType.Copy
    sb, ps, psd = pools["sb"], pools["ps"], pools["psd"]
    m64, mb, wv32, wvb, q0b = (pools[k] for k in ("m64", "mb", "wv32", "wvb", "q0b"))
    qM = pools["qM"][b]
    pTb, pT8 = pools["pTb"][b], pools["pT8"][b]
    vpb, vp8 = pools["vpb"][b], pools["vp8"][b]

    # -- A: qM = query @ M.  fp8 DoubleRow for cols t>=128; weight DMAs for
    #    batch 0 interleave chunk-by-chunk with the input loads so each
    #    matmul's operands arrive together. --
    qT = sb.tile([P, NC, T], F8, name="qT")
    for ec in range(NC):
        if b == 0:
            nc.sync.dma_start(m64[:, ec, :], dram["m64"][bass.ts(ec, P), :])
        nc.sync.dma_start(qT[:, ec, :], dram["qT"][b, bass.ts(ec, P), :])
    if b == 0:
        for ec in range(NC):
            nc.sync.dma_start(mb[:, ec, :], dram["mb"][bass.ts(ec, P), :])
            nc.sync.dma_start(q0b[:, ec, :], dram["q0b"][bass.ts(ec, P), :])
    for ec2 in range(NC):
        for lo, w in ((P, 384), (512, 512)):
            acc = ps.tile([P, 512], F32, name="ps")
            for j in range(4):
                nc.tensor.matmul(
                    acc[:, :w],
                    lhsT=m64[:, 2 * j:2 * j + 2, bass.ts(ec2, P)],
                    rhs=qT[:, 2 * j:2 * j + 2, lo:lo + w],
                    start=(j == 0),
                    stop=(j == 3),
                    perf_mode=DR,
                )
            nc.scalar.activation(qM[:, ec2, lo:lo + w], acc[:, :w], Copy,
                                 scale=1.0 / 64.0)
    # A-fixup: bf16 qM cols t<128, both batches at once (weights shared)
    if b == 0:
        for ec2 in range(NC):
            acc = ps.tile([P, 512], F32, name="ps")
            for ec1 in range(NC):
                nc.tensor.matmul(
                    acc[:, :BPC * P],
                    lhsT=mb[:, ec1, bass.ts(ec2, P)],
                    rhs=q0b[:, ec1, :],
                    start=(ec1 == 0),
                    stop=(ec1 == NC - 1),
                )
            for bb in range(BPC):
                nc.scalar.copy(pools["qM"][bb][:, ec2, 0:P],
                               acc[:, bb * P:(bb + 1) * P])

    # -- B: scoresT[s,t] = kT-chunk.T @ qM-chunks (bf16), exp, causal zero.
    #    p stored bf16 for s<256 (protects the early-row fixup), fp8 above
    #    (weight noise is self-normalized by den built from the same values).
    #    exp is shifted by -2 so p_max ~ e^4 stays under fp8's 240 max; the
    #    shift cancels exactly in the num/den normalization. --
    kT = sb.tile([P, NC, T], BF16, name="kT")
    for ec in range(NC):
        nc.sync.dma_start(kT[:, ec, :], dram["kT"][b, bass.ts(ec, P), :])
    padt = sb.tile([P, NC], BF16, name="padt", bufs=2)
    nc.sync.dma_start(
        padt[:], dram["pad"][b].rearrange("(c p) x -> p (c x)", p=P)
    )
    # 256-wide t-groups: 23 causal-live [128s x 256t] blocks vs 13 512-wide
    # ones (26 512-equivalents) — ~12% less B stream time.
    GW = 256
    denrow = sb.tile([1, T], F32, name="denrow", bufs=2)
    for g in range(T // GW):
        nlive = min(2 * (g + 1) + 1, NC)
        dsts = []
        for sc in range(nlive):
            acc = ps.tile([P, GW], F32, name="ps")
            for ec2 in range(NC):
                nc.tensor.matmul(
                    acc[:],
                    lhsT=kT[:, ec2, bass.ts(sc, P)],
                    rhs=qM[:, ec2, bass.ts(g, GW)],
                    start=(ec2 == 0),
                    stop=(ec2 == NC - 1),
                )
            if sc < 2:
                dst = pTb[:, sc, bass.ts(g, GW)]
            else:
                dst = pT8[:, sc - 2, bass.ts(g, GW)]
            nc.scalar.activation(dst, acc[:], Exp, bias=pools["nbias"][:])
            off = P * sc - GW * g
            if off >= -126:
                # keep where t_local - s_local - off + 1 >= 0 (j <= i+1)
                nc.gpsimd.affine_select(
                    out=dst,
                    in_=dst,
                    compare_op=mybir.AluOpType.is_ge,
                    fill=0.0,
                    base=1 - off,
                    pattern=[[1, GW]],
                    channel_multiplier=-1,
                )
            dsts.append(dst)
        # den row = sum of stored p (post-select) on the PE: 1-partition
        # outputs riding wide streams, replacing 86 one-col matmuls
        # (~13us exposed).  Emitted after the whole block loop so the PE
        # queue never parks waiting for a block's exp+select round-trip.
        denr = psd.tile([1, GW], F32, name="psd")
        for sc, dst in enumerate(dsts):
            nc.tensor.matmul(denr[:], lhsT=padt[:, sc:sc + 1], rhs=dst,
                             start=(sc == 0), stop=(sc == nlive - 1))
        nc.scalar.copy(denrow[:, bass.ts(g, GW)], denr[:])
    # transpose den [1,T] -> [t-partition, chunk] via 8 tiny DMAs, then recip
    dencol = sb.tile([P, NC], F32, name="dencol", bufs=2)
    for c in range(NC):
        nc.sync.dma_start(dencol[:, c:c + 1], denrow[:, bass.ts(c, P)])
    rcol = sb.tile([P, NC], F32, name="rcol", bufs=2)
    nc.vector.reciprocal(rcol[:], dencol[:])

    # -- V: v'[s,h] = value @ Wv.T.  fp8 DoubleRow for s>=128 --
    vT = sb.tile([P, NC, T], F8, name="vT")
    for ec in range(NC):
        if b == 0:
            nc.sync.dma_start(wv32[:, ec, :], dram["wv32"][bass.ts(ec, P), :])
        nc.sync.dma_start(vT[:, ec, :], dram["vT"][b, bass.ts(ec, P), :])
    v0b = sb.tile([P, NC, P], BF16, name="v0b")
    for ec in range(NC):
        if b == 0:
            nc.sync.dma_start(wvb[:, ec, :], dram["wvb"][bass.ts(ec, P), :])
        nc.sync.dma_start(v0b[:, ec, :], dram["v0b"][b, bass.ts(ec, P), :])
    for sc in range(1, NC):
        for hg in range(2):
            acc = ps.tile([P, 512], F32, name="ps")
            for j in range(4):
                nc.tensor.matmul(
                    acc[:],
                    lhsT=vT[:, 2 * j:2 * j + 2, bass.ts(sc, P)],
                    rhs=wv32[:, 2 * j:2 * j + 2, bass.ts(hg, 512)],
                    start=(j == 0),
                    stop=(j == 3),
                    perf_mode=DR,
                )
            if sc < 2:
                dst = vpb[:, sc, bass.ts(hg, 512)]
            else:
                dst = vp8[:, sc - 2, bass.ts(hg, 512)]
            nc.vector.tensor_scalar_mul(dst, acc[:], 1.0 / 32.0)
    # V-fixup: bf16 v' rows s<128
    for hg in range(2):
        acc = ps.tile([P, 512], F32, name="ps")
        for ec in range(NC):
            nc.tensor.matmul(
                acc[:],
                lhsT=v0b[:, ec, :],
                rhs=wvb[:, ec, bass.ts(hg, 512)],
                start=(ec == 0),
                stop=(ec == NC - 1),
            )
        nc.vector.tensor_copy(vpb[:, 0, bass.ts(hg, 512)], acc[:])

    # -- C': attn[t,h] = sum_s p[t,s] v'[s,h]; s<256 bf16, above fp8 pairs --
    for ti in range(NC):
        nsc = min(ti + 2, NC)
        po0 = ps.tile([P, 512], F32, name="ps")
        po1 = ps.tile([P, 512], F32, name="ps")
        c8 = nsc - 2  # fp8 s-chunks
        for sc in range(min(nsc, 2)):
            lhsT = pTb[:, sc, bass.ts(ti, P)]
            st, sp = (sc == 0), (sc == nsc - 1)
            nc.tensor.matmul(po0[:], lhsT=lhsT, rhs=vpb[:, sc, 0:512],
                             start=st, stop=sp)
            nc.tensor.matmul(po1[:], lhsT=lhsT, rhs=vpb[:, sc, 512:1024],
                             start=st, stop=sp)
        for j in range(c8 // 2):
            lhsT = pT8[:, 2 * j:2 * j + 2, bass.ts(ti, P)]
            sp = (2 * j + 2 == c8)
            nc.tensor.matmul(po0[:], lhsT=lhsT, rhs=vp8[:, 2 * j:2 * j + 2, 0:512],
                             start=False, stop=sp, perf_mode=DR)
            nc.tensor.matmul(po1[:], lhsT=lhsT,
                             rhs=vp8[:, 2 * j:2 * j + 2, 512:1024],
                             start=False, stop=sp, perf_mode=DR)
        if c8 > 0 and c8 % 2:
            lhsT = pT8[:, c8 - 1, bass.ts(ti, P)]
            nc.tensor.matmul(po0[:], lhsT=lhsT, rhs=vp8[:, c8 - 1, 0:512],
                             start=False, stop=True)
            nc.tensor.matmul(po1[:], lhsT=lhsT, rhs=vp8[:, c8 - 1, 512:1024],
                             start=False, stop=True)
        osb = sb.tile([P, T], BF16, name="osb", bufs=3)
        # the two halves scale concurrently on VectorE and ScalarE
        nc.vector.tensor_scalar_mul(osb[:, 0:512], po0[:], rcol[:, ti:ti + 1])
        nc.sync.dma_start(dram["out"][b, bass.ts(ti, P), 0:512], osb[:, 0:512])
        nc.scalar.activation(osb[:, 512:1024], po1[:], Copy,
                             scale=rcol[:, ti:ti + 1])
        nc.sync.dma_start(dram["out"][b, bass.ts(ti, P), 512:1024],
                          osb[:, 512:1024])


def _build_nc():
    nc = bass.Bass()
    dram = {
        "qT": nc.declare_dram_parameter("qT", [BPC, E, T], F8, isOutput=False),
        "kT": nc.declare_dram_parameter("kT", [BPC, E, T], BF16, isOutput=False),
        "vT": nc.declare_dram_parameter("vT", [BPC, E, T], F8, isOutput=False),
        "q0b": nc.declare_dram_parameter("q0b", [E, BPC * P], BF16, isOutput=False),
        "v0b": nc.declare_dram_parameter("v0b", [BPC, E, P], BF16, isOutput=False),
        "m64": nc.declare_dram_parameter("m64", [E, E], F8, isOutput=False),
        "mb": nc.declare_dram_parameter("mb", [E, E], BF16, isOutput=False),
        "wv32": nc.declare_dram_parameter("wv32", [E, H], F8, isOutput=False),
        "wvb": nc.declare_dram_parameter("wvb", [E, H], BF16, isOutput=False),
        "pad": nc.declare_dram_parameter("pad", [BPC, T, 1], BF16, isOutput=False),
        "out": nc.declare_dram_parameter("out", [BPC, T, H], BF16, isOutput=True),
    }
    with tile.TileContext(nc) as tc, ExitStack() as ctx:
        sb = ctx.enter_context(tc.tile_pool(name="sb", bufs=1))
        ps = ctx.enter_context(tc.tile_pool(name="ps", bufs=7, space="PSUM"))
        psd = ctx.enter_context(tc.tile_pool(name="psd", bufs=1, space="PSUM"))

        pools = {"sb": sb, "ps": ps, "psd": psd}
        pools["m64"] = sb.tile([P, NC, E], F8, name="m64")
        pools["mb"] = sb.tile([P, NC, E], BF16, name="mb")
        pools["wv32"] = sb.tile([P, NC, H], F8, name="wv32")
        pools["wvb"] = sb.tile([P, NC, H], BF16, name="wvb")
        pools["q0b"] = sb.tile([P, NC, BPC * P], BF16, name="q0b")
        pools["qM"] = [sb.tile([P, NC, T], BF16, name=f"qM{b}") for b in range(BPC)]
        pools["pTb"] = [sb.tile([P, 2, T], BF16, name=f"pTb{b}") for b in range(BPC)]
        pools["pT8"] = [sb.tile([P, NC - 2, T], F8, name=f"pT8{b}") for b in range(BPC)]
        pools["vpb"] = [sb.tile([P, 2, H], BF16, name=f"vpb{b}") for b in range(BPC)]
        pools["vp8"] = [sb.tile([P, NC - 2, H], F8, name=f"vp8{b}") for b in range(BPC)]

        # PE warm-up: ~3.4us of junk matmuls with no data dependencies fill
        # the startup DMA window and trip the HAM clock gate to 2.4 GHz
        # before the first real matmul arrives.  Output is never read.
        pools["nbias"] = sb.tile([P, 1], F32, name="nbias")
        nc.gpsimd.memset(pools["nbias"][:], -2.0)
        warm = sb.tile([P, 512], BF16, name="warm")
        nc.vector.memset(warm[:], 0.0)
        wps = ps.tile([P, 512], F32, name="ps")
        for _ in range(16):
            nc.tensor.matmul(wps[:], lhsT=warm[:, 0:P], rhs=warm[:],
                             start=True, stop=True)

        for b in range(BPC):
            _emit_batch(nc, pools, b, dram)

    _split_multi_waits(nc)
    return nc


def _get_nc():
    global _nc_cache
    if _nc_cache is None:
        _nc_cache = _build_nc()
    return _nc_cache


def _make_in_maps(key, query, value, padding_mask, Wk, Wq, Wv):
    bf = ml_dtypes.bfloat16
    f8 = ml_dtypes.float8_e4m3
    M = (Wq.T.astype(np.float64) @ Wk.astype(np.float64)
         / np.sqrt(np.float64(E))).astype(np.float32)
    m64 = (M * 64.0).astype(f8)
    mb = M.astype(bf)
    wv32 = (Wv.T * 32.0).astype(f8)
    wvb = np.ascontiguousarray(Wv.T).astype(bf)
    pad01 = (padding_mask.reshape(NB, T) == 0).astype(np.float32)  # [B,T]
    in_maps = []
    for c in range(NCORES):
        s = slice(BPC * c, BPC * (c + 1))
        qTf = query[s].transpose(0, 2, 1)
        kTf = key[s].transpose(0, 2, 1)
        vTf = value[s].transpose(0, 2, 1) * pad01[s][:, None, :]
        q0b = np.ascontiguousarray(
            qTf[:, :, :P].transpose(1, 0, 2).reshape(E, BPC * P)).astype(bf)
        in_maps.append({
            "qT": np.ascontiguousarray(qTf).astype(f8),
            "kT": np.ascontiguousarray(kTf).astype(bf),
            "vT": np.ascontiguousarray(vTf).astype(f8),
            "q0b": q0b,
            "v0b": np.ascontiguousarray(vTf[:, :, :P]).astype(bf),
            "m64": m64, "mb": mb, "wv32": wv32, "wvb": wvb,
            "pad": pad01[s].astype(bf).reshape(BPC, T, 1),
        })
    return in_maps


def run_on_cores(in_maps, trace=False, **kw):
    nc = _get_nc()
    return run_bass_kernel_spmd(nc, in_maps, list(range(NCORES)), trace=trace, **kw)


def kernel(key, query, value, padding_mask, Wk, Wq, Wv):
    key = np.asarray(key)
    query = np.asarray(query)
    value = np.asarray(value)
    padding_mask = np.asarray(padding_mask)
    in_maps = _make_in_maps(key, query, value, padding_mask,
                            np.asarray(Wk), np.asarray(Wq), np.asarray(Wv))
    res = run_on_cores(in_maps)
    out = np.empty((NB, T, H), np.float32)
    for c in range(NCORES):
        out[BPC * c: BPC * (c + 1)] = res.results[c]["out"].astype(np.float32)
    return out


# revision 23
# speedup vs baseline: 1.1223x; 1.0256x over previous
"""Causal (diagonal=1) attention head for trn2, 8-core SPMD, fp8-hybrid.

Reference computation (fp32):
    k = key @ Wk.T; q = query @ Wq.T; v = value @ Wv.T       # [B,T,H]
    qk = (q @ k.T) / sqrt(E)                                  # [B,T,T]
    qk masked with tril(ones, k=1) and padding_mask           # -inf outside
    attn = softmax(qk, -1) @ v                                # [B,T,H]

Algebraic fold (removes one of three projections):
    qk = query @ M @ key.T   with  M = Wq.T @ Wk / sqrt(E)  (host, fp64)
    attn = softmax-normalized p @ (value @ Wv.T)

Sharding: data-parallel over batch, 2 batches per core, no collectives.

Per-core pipeline (per batch), PSUM always fp32:
    A : qM = query @ (64*M)  fp8 DoubleRow, requant bf16 with scale 1/64.
        Rows t<128 instead use a bf16 matmul (bf16 M) — the softmax of
        early rows has few live keys, so quantization noise there is not
        averaged away; everywhere else fp8 noise cancels across keys.
    B : scores = qM @ keyT   bf16 (exp() amplifies score noise; fp8 here
        fails the 2e-2 gate — measured 4.4e-2 vs 1.1e-2 in simulation)
    p = exp(scores) (ScalarE), causal-zeroed via GPSIMD affine_select
    V : v' = value @ (32*Wv.T) fp8 DoubleRow, requant bf16 scale 1/32;
        rows s<128 via bf16 matmul (same early-row argument).
    C': num = p @ v' ; den = p @ pad01   bf16
    out = num * reciprocal(den)  -> bf16 DMA (cast fp32 on host)

padding_mask is folded in exactly on the host: value rows and the
denominator column are scaled by pad01 = (padding_mask == 0), which
equals softmax with -inf at padded keys.
"""
from contextlib import ExitStack

import numpy as np
import ml_dtypes

import concourse.bass as bass
import concourse.mybir as mybir
import concourse.tile as tile
from concourse.bass_utils import run_bass_kernel_spmd

BF16 = mybir.dt.bfloat16
F8 = mybir.dt.float8e4
F32 = mybir.dt.float32
DR = mybir.MatmulPerfMode.DoubleRow
P = 128
T = 1024           # sequence length
E = 1024           # embed dim
H = 1024           # head dim
NB = 16            # full batch
NCORES = 8
BPC = NB // NCORES  # batches per core
NC = T // P        # 128-chunks per dim (8)

_nc_cache = None


# --- walrus workaround: one sync-wait per instruction ---------------------
def _split_multi_waits(nc):
    """This walrus build rejects instructions with >1 sync wait (2 for
    EventSemaphore).  Move extra waits onto fresh same-engine NOPs placed
    immediately before the instruction; per-engine in-order execution
    preserves the gating, and semaphore updates stay on the original."""
    for fn in nc.m.functions:
        for bb in fn.blocks:
            il = bb.instructions
            idx = 0
            while idx < len(il):
                inst = il[idx]
                si = inst.sync_info
                waits = list(si.on_wait) if si and si.on_wait else []
                cap = 2 if isinstance(inst, mybir.InstEventSemaphore) else 1
                if len(waits) > cap:
                    extra, keep = waits[:-cap], waits[-cap:]
                    for j, w in enumerate(extra):
                        nop = mybir.InstNoOp(
                            name=f"I-wsplit-{inst.name}-{j}",
                            engine=inst.engine,
                            ins=[],
                            outs=[],
                            sync_info=mybir.SyncInfo(on_wait=[w], on_update=[]),
                        )
                        il.insert(idx, nop)
                        idx += 1
                    inst.sync_info = mybir.SyncInfo(
                        on_wait=keep, on_update=list(si.on_update or [])
                    )
                idx += 1


def _emit_batch(nc, pools, b, dram):
    Exp = mybir.ActivationFunctionType.Exp
    Copy = mybir.ActivationFunctionType.Copy
    sb, ps, psd = pools["sb"], pools["ps"], pools["psd"]
    m64, mb, wv32, wvb, q0b = (pools[k] for k in ("m64", "mb", "wv32", "wvb", "q0b"))
    qM = pools["qM"][b]
    pTb, pT8 = pools["pTb"][b], pools["pT8"][b]
    vpb, vp8 = pools["vpb"][b], pools["vp8"][b]

    # -- A: qM = query @ M.  fp8 DoubleRow for cols t>=128; weight DMAs for
    #    batch 0 interleave chunk-by-chunk with the input loads so each
    #    matmul's operands arrive together. --
    qT = sb.tile([P, NC, T], F8, name="qT")
    for ec in range(NC):
        if b == 0:
            nc.sync.dma_start(m64[:, ec, :], dram["m64"][bass.ts(ec, P), :])
        nc.sync.dma_start(qT[:, ec, :], dram["qT"][b, bass.ts(ec, P), :])
    if b == 0:
        for ec in range(NC):
            nc.sync.dma_start(mb[:, ec, :], dram["mb"][bass.ts(ec, P), :])
            nc.sync.dma_start(q0b[:, ec, :], dram["q0b"][bass.ts(ec, P), :])
    for ec2 in range(NC):
        for lo, w in ((P, 384), (512, 512)):
            acc = ps.tile([P, 512], F32, name="ps")
            for j in range(4):
                nc.tensor.matmul(
                    acc[:, :w],
                    lhsT=m64[:, 2 * j:2 * j + 2, bass.ts(ec2, P)],
                    rhs=qT[:, 2 * j:2 * j + 2, lo:lo + w],
                    start=(j == 0),
                    stop=(j == 3),
                    perf_mode=DR,
                )
            nc.scalar.activation(qM[:, ec2, lo:lo + w], acc[:, :w], Copy,
                                 scale=1.0 / 64.0)
    # A-fixup: bf16 qM cols t<128, both batches at once (weights shared)
    if b == 0:
        for ec2 in range(NC):
            acc = ps.tile([P, 512], F32, name="ps")
            for ec1 in range(NC):
                nc.tensor.matmul(
                    acc[:, :BPC * P],
                    lhsT=mb[:, ec1, bass.ts(ec2, P)],
                    rhs=q0b[:, ec1, :],
                    start=(ec1 == 0),
                    stop=(ec1 == NC - 1),
                )
            for bb in range(BPC):
                nc.scalar.copy(pools["qM"][bb][:, ec2, 0:P],
                               acc[:, bb * P:(bb + 1) * P])

    # -- B: scoresT[s,t] = kT-chunk.T @ qM-chunks (bf16), exp, causal zero.
    #    p stored bf16 for s<128 (protects the early-row fixup), fp8 above
    #    (weight noise is self-normalized by den built from the same values).
    #    exp is shifted by -2 so p_max ~ e^4 stays under fp8's 240 max; the
    #    shift cancels exactly in the num/den normalization. --
    kT = sb.tile([P, NC, T], BF16, name="kT")
    for ec in range(NC):
        nc.sync.dma_start(kT[:, ec, :], dram["kT"][b, bass.ts(ec, P), :])
    padt = sb.tile([P, NC], BF16, name="padt", bufs=2)
    nc.sync.dma_start(
        padt[:], dram["pad"][b].rearrange("(c p) x -> p (c x)", p=P)
    )
    # 256-wide t-groups: 23 causal-live [128s x 256t] blocks vs 13 512-wide
    # ones (26 512-equivalents) — ~12% less B stream time.
    GW = 256
    denrow = sb.tile([1, T], F32, name="denrow", bufs=2)
    for g in range(T // GW):
        nlive = min(2 * (g + 1) + 1, NC)
        dsts = []
        for sc in range(nlive):
            acc = ps.tile([P, GW], F32, name="ps")
            for ec2 in range(NC):
                nc.tensor.matmul(
                    acc[:],
                    lhsT=kT[:, ec2, bass.ts(sc, P)],
                    rhs=qM[:, ec2, bass.ts(g, GW)],
                    start=(ec2 == 0),
                    stop=(ec2 == NC - 1),
                )
            if sc < 1:
                dst = pTb[:, sc, bass.ts(g, GW)]
            else:
                dst = pT8[:, sc - 1, bass.ts(g, GW)]
            nc.scalar.activation(dst, acc[:], Exp, bias=pools["nbias"][:])
            off = P * sc - GW * g
            if off >= -126:
                # keep where t_local - s_local - off + 1 >= 0 (j <= i+1)
                nc.gpsimd.affine_select(
                    out=dst,
                    in_=dst,
                    compare_op=mybir.AluOpType.is_ge,
                    fill=0.0,
                    base=1 - off,
                    pattern=[[1, GW]],
                    channel_multiplier=-1,
                )
            dsts.append(dst)
        # den row = sum of stored p (post-select) on the PE: 1-partition
        # outputs riding wide streams, replacing 86 one-col matmuls
        # (~13us exposed).  Emitted after the whole block loop so the PE
        # queue never parks waiting for a block's exp+select round-trip.
        denr = psd.tile([1, GW], F32, name="psd")
        for sc, dst in enumerate(dsts):
            nc.tensor.matmul(denr[:], lhsT=padt[:, sc:sc + 1], rhs=dst,
                             start=(sc == 0), stop=(sc == nlive - 1))
        nc.scalar.copy(denrow[:, bass.ts(g, GW)], denr[:])
    # transpose den [1,T] -> [t-partition, chunk] via 8 tiny DMAs, then recip
    dencol = sb.tile([P, NC], F32, name="dencol", bufs=2)
    for c in range(NC):
        nc.sync.dma_start(dencol[:, c:c + 1], denrow[:, bass.ts(c, P)])
    rcol = sb.tile([P, NC], F32, name="rcol", bufs=2)
    nc.vector.reciprocal(rcol[:], dencol[:])

    # -- V: v'[s,h] = value @ Wv.T.  fp8 DoubleRow for s>=128 --
    vT = sb.tile([P, NC, T], F8, name="vT")
    for ec in range(NC):
        if b == 0:
            nc.sync.dma_start(wv32[:, ec, :], dram["wv32"][bass.ts(ec, P), :])
        nc.sync.dma_start(vT[:, ec, :], dram["vT"][b, bass.ts(ec, P), :])
    v0b = sb.tile([P, NC, P], BF16, name="v0b")
    for ec in range(NC):
        if b == 0:
            nc.sync.dma_start(wvb[:, ec, :], dram["wvb"][bass.ts(ec, P), :])
        nc.sync.dma_start(v0b[:, ec, :], dram["v0b"][b, bass.ts(ec, P), :])
    for sc in range(1, NC):
        for hg in range(2):
            acc = ps.tile([P, 512], F32, name="ps")
            for j in range(4):
                nc.tensor.matmul(
                    acc[:],
                    lhsT=vT[:, 2 * j:2 * j + 2, bass.ts(sc, P)],
                    rhs=wv32[:, 2 * j:2 * j + 2, bass.ts(hg, 512)],
                    start=(j == 0),
                    stop=(j == 3),
                    perf_mode=DR,
                )
            dst = vp8[:, sc - 1, bass.ts(hg, 512)]
            nc.vector.tensor_scalar_mul(dst, acc[:], 1.0 / 32.0)
    # V-fixup: bf16 v' rows s<128
    for hg in range(2):
        acc = ps.tile([P, 512], F32, name="ps")
        for ec in range(NC):
            nc.tensor.matmul(
                acc[:],
                lhsT=v0b[:, ec, :],
                rhs=wvb[:, ec, bass.ts(hg, 512)],
                start=(ec == 0),
                stop=(ec == NC - 1),
            )
        nc.vector.tensor_copy(vpb[:, 0, bass.ts(hg, 512)], acc[:])

    # -- C': attn[t,h] = sum_s p[t,s] v'[s,h]; s<128 bf16, above fp8 pairs --
    for ti in range(NC):
        nsc = min(ti + 2, NC)
        po0 = ps.tile([P, 512], F32, name="ps")
        po1 = ps.tile([P, 512], F32, name="ps")
        c8 = nsc - 1  # fp8 s-chunks
        for sc in range(1):
            lhsT = pTb[:, sc, bass.ts(ti, P)]
            st, sp = (sc == 0), (sc == nsc - 1)
            nc.tensor.matmul(po0[:], lhsT=lhsT, rhs=vpb[:, sc, 0:512],
                             start=st, stop=sp)
            nc.tensor.matmul(po1[:], lhsT=lhsT, rhs=vpb[:, sc, 512:1024],
                             start=st, stop=sp)
        for j in range(c8 // 2):
            lhsT = pT8[:, 2 * j:2 * j + 2, bass.ts(ti, P)]
            sp = (2 * j + 2 == c8)
            nc.tensor.matmul(po0[:], lhsT=lhsT, rhs=vp8[:, 2 * j:2 * j + 2, 0:512],
                             start=False, stop=sp, perf_mode=DR)
            nc.tensor.matmul(po1[:], lhsT=lhsT,
                             rhs=vp8[:, 2 * j:2 * j + 2, 512:1024],
                             start=False, stop=sp, perf_mode=DR)
        if c8 > 0 and c8 % 2:
            lhsT = pT8[:, c8 - 1, bass.ts(ti, P)]
            nc.tensor.matmul(po0[:], lhsT=lhsT, rhs=vp8[:, c8 - 1, 0:512],
                             start=False, stop=True)
            nc.tensor.matmul(po1[:], lhsT=lhsT, rhs=vp8[:, c8 - 1, 512:1024],
                             start=False, stop=True)
        osb = sb.tile([P, T], BF16, name="osb", bufs=3)
        # the two halves scale concurrently on VectorE and ScalarE
        nc.vector.tensor_scalar_mul(osb[:, 0:512], po0[:], rcol[:, ti:ti + 1])
        nc.sync.dma_start(dram["out"][b, bass.ts(ti, P), 0:512], osb[:, 0:512])
        nc.scalar.activation(osb[:, 512:1024], po1[:], Copy,
                             scale=rcol[:, ti:ti + 1])
        nc.sync.dma_start(dram["out"][b, bass.ts(ti, P), 512:1024],
                          osb[:, 512:1024])


def _build_nc():
    nc = bass.Bass()
    dram = {
        "qT": nc.declare_dram_parameter("qT", [BPC, E, T], F8, isOutput=False),
        "kT": nc.declare_dram_parameter("kT", [BPC, E, T], BF16, isOutput=False),
        "vT": nc.declare_dram_parameter("vT", [BPC, E, T], F8, isOutput=False),
        "q0b": nc.declare_dram_parameter("q0b", [E, BPC * P], BF16, isOutput=False),
        "v0b": nc.declare_dram_parameter("v0b", [BPC, E, P], BF16, isOutput=False),
        "m64": nc.declare_dram_parameter("m64", [E, E], F8, isOutput=False),
        "mb": nc.declare_dram_parameter("mb", [E, E], BF16, isOutput=False),
        "wv32": nc.declare_dram_parameter("wv32", [E, H], F8, isOutput=False),
        "wvb": nc.declare_dram_parameter("wvb", [E, H], BF16, isOutput=False),
        "pad": nc.declare_dram_parameter("pad", [BPC, T, 1], BF16, isOutput=False),
        "out": nc.declare_dram_parameter("out", [BPC, T, H], BF16, isOutput=True),
    }
    with tile.TileContext(nc) as tc, ExitStack() as ctx:
        sb = ctx.enter_context(tc.tile_pool(name="sb", bufs=1))
        ps = ctx.enter_context(tc.tile_pool(name="ps", bufs=7, space="PSUM"))
        psd = ctx.enter_context(tc.tile_pool(name="psd", bufs=1, space="PSUM"))

        pools = {"sb": sb, "ps": ps, "psd": psd}
        pools["m64"] = sb.tile([P, NC, E], F8, name="m64")
        pools["mb"] = sb.tile([P, NC, E], BF16, name="mb")
        pools["wv32"] = sb.tile([P, NC, H], F8, name="wv32")
        pools["wvb"] = sb.tile([P, NC, H], BF16, name="wvb")
        pools["q0b"] = sb.tile([P, NC, BPC * P], BF16, name="q0b")
        pools["qM"] = [sb.tile([P, NC, T], BF16, name=f"qM{b}") for b in range(BPC)]
        pools["pTb"] = [sb.tile([P, 1, T], BF16, name=f"pTb{b}") for b in range(BPC)]
        pools["pT8"] = [sb.tile([P, NC - 1, T], F8, name=f"pT8{b}") for b in range(BPC)]
        pools["vpb"] = [sb.tile([P, 1, H], BF16, name=f"vpb{b}") for b in range(BPC)]
        pools["vp8"] = [sb.tile([P, NC - 1, H], F8, name=f"vp8{b}") for b in range(BPC)]

        # PE warm-up: ~3.4us of junk matmuls with no data dependencies fill
        # the startup DMA window and trip the HAM clock gate to 2.4 GHz
        # before the first real matmul arrives.  Output is never read.
        pools["nbias"] = sb.tile([P, 1], F32, name="nbias")
        nc.gpsimd.memset(pools["nbias"][:], -2.0)
        warm = sb.tile([P, 512], BF16, name="warm")
        nc.vector.memset(warm[:], 0.0)
        wps = ps.tile([P, 512], F32, name="ps")
        for _ in range(16):
            nc.tensor.matmul(wps[:], lhsT=warm[:, 0:P], rhs=warm[:],
                             start=True, stop=True)

        for b in range(BPC):
            _emit_batch(nc, pools, b, dram)

    _split_multi_waits(nc)
    return nc


def _get_nc():
    global _nc_cache
    if _nc_cache is None:
        _nc_cache = _build_nc()
    return _nc_cache


def _make_in_maps(key, query, value, padding_mask, Wk, Wq, Wv):
    bf = ml_dtypes.bfloat16
    f8 = ml_dtypes.float8_e4m3
    M = (Wq.T.astype(np.float64) @ Wk.astype(np.float64)
         / np.sqrt(np.float64(E))).astype(np.float32)
    m64 = (M * 64.0).astype(f8)
    mb = M.astype(bf)
    wv32 = (Wv.T * 32.0).astype(f8)
    wvb = np.ascontiguousarray(Wv.T).astype(bf)
    pad01 = (padding_mask.reshape(NB, T) == 0).astype(np.float32)  # [B,T]
    in_maps = []
    for c in range(NCORES):
        s = slice(BPC * c, BPC * (c + 1))
        qTf = query[s].transpose(0, 2, 1)
        kTf = key[s].transpose(0, 2, 1)
        vTf = value[s].transpose(0, 2, 1) * pad01[s][:, None, :]
        q0b = np.ascontiguousarray(
            qTf[:, :, :P].transpose(1, 0, 2).reshape(E, BPC * P)).astype(bf)
        in_maps.append({
            "qT": np.ascontiguousarray(qTf).astype(f8),
            "kT": np.ascontiguousarray(kTf).astype(bf),
            "vT": np.ascontiguousarray(vTf).astype(f8),
            "q0b": q0b,
            "v0b": np.ascontiguousarray(vTf[:, :, :P]).astype(bf),
            "m64": m64, "mb": mb, "wv32": wv32, "wvb": wvb,
            "pad": pad01[s].astype(bf).reshape(BPC, T, 1),
        })
    return in_maps


def run_on_cores(in_maps, trace=False, **kw):
    nc = _get_nc()
    return run_bass_kernel_spmd(nc, in_maps, list(range(NCORES)), trace=trace, **kw)


def kernel(key, query, value, padding_mask, Wk, Wq, Wv):
    key = np.asarray(key)
    query = np.asarray(query)
    value = np.asarray(value)
    padding_mask = np.asarray(padding_mask)
    in_maps = _make_in_maps(key, query, value, padding_mask,
                            np.asarray(Wk), np.asarray(Wq), np.asarray(Wv))
    res = run_on_cores(in_maps)
    out = np.empty((NB, T, H), np.float32)
    for c in range(NCORES):
        out[BPC * c: BPC * (c + 1)] = res.results[c]["out"].astype(np.float32)
    return out
